# revision 45
# baseline (speedup 1.0000x reference)
"""Trainium2 Bass kernel for nn_Attention_85796266705382.

Reference computation (per batch element, b=8, HEAD=8, n=32*32=1024, c=dim=512):
    qkv = x @ w_qkv                      # (n, 1536), per-head interleaved [q|k|v] x 64
    q,k,v per head (n, 64)
    attn = softmax(q @ k.T * 8**-0.5)    # scale uses FULL batch size (reference quirk)
    out  = attn @ v                      # (n, 64) per head -> (n, 512)
    y    = out @ w_out + b_out           # (n, 512)

Sharding: pure data-parallel over batch - one batch element per NeuronCore (8 cores).

Per-core design (v2): the kernel is scheduled as a continuous software pipeline
around the two nearly co-critical engines: PE (~82us of matmul rows at 2.4GHz)
and ACT (64 exps of [128,1024] ~ 69us). The pipelined unit is one (j-tile, head)
half-slot: 2 QK matmuls -> exp -> pt tile. All other PE work (QKV projections,
v-pass, AV matmuls of the PREVIOUS pair, final projection chunks) is emitted in
small chunks between the QK matmuls so the exp stream never starves and PE never
idles waiting on exp.

Key layout/scheduling choices:
  * qkT [1024, n] = w_qk.T @ xT with host-permuted pair-banded columns
    [q_h0 q_h1 k_h0 k_h1 | ...] -> each 128-row tile is a head-PAIR band.
  * scores (transposed) sT_h [j, i] per (pair, jt, head): 2 matmuls (i-chunks of
    512), one [128,1024] exp on ACT (folds the 8**-0.5 scale), bf16 pt output.
  * v natural [n, 512] via f32r matmuls (no bf16 cast of xT needed); stored
    bf16 with 65-column per-head pitch, col 64 = ones -> AV matmul emits the
    softmax denominators for free in PSUM row 64.
  * AV per pair split into two i-chunk phases of [65,512] accumulators so only
    2 PSUM banks are held: scores 2x[128,1024] (4 banks) + AV 2x[65,512] (2) +
    projection transient [128,1024] (2) = 8 banks exactly.
  * normalization fully on-chip: DVE reciprocal of PSUM den row -> gpsimd
    partition_broadcast -> DVE multiply into ot[p] (no DRAM bounce).
  * final projection y = ot.T @ w_out accumulated over pairs in PSUM at the
    end; bias add on DVE; stores alternate sync/scalar DMA queues.
"""

import numpy as np


def _ensure_paths():
    import sys

    try:
        import concourse.bass  # noqa: F401

        return
    except ImportError:
        pass
    for p in ("/opt/trn_rl_repo", "/root/.axon_site/_ro/trn_rl_repo"):
        if p not in sys.path:
            sys.path.append(p)
    import concourse.bass  # noqa: F401


HEAD = 8
B = 8
N = 1024  # tokens per batch element (32*32)
C = 512  # channels
DIM = 512
DH = 64
SCALE = float(B) ** -0.5  # reference scales by batch size, reproduced faithfully
N_CORES = 8

_CACHE = {}


def _split_excess_waits(nc, mybir, bass_rust):
    """walrus in this container accepts 1 sync wait per instruction (2 for
    EventSemaphore); Tile sometimes attaches more. Hoist the excess onto fresh
    same-engine NoOps inserted just before the over-capacity instruction."""
    n_split = 0
    for fn in nc.m.functions:
        for bb in fn.blocks:
            insts = bb.instructions
            i = 0
            while i < len(insts):
                inst = insts[i]
                si = inst.sync_info
                cap = 2 if isinstance(inst, mybir.InstEventSemaphore) else 1
                if si is not None and len(si.on_wait) > cap:
                    extra = list(si.on_wait[cap:])
                    del si.on_wait[cap:]
                    new_insts = []
                    for k in range(0, len(extra), 2):
                        pair = extra[k : k + 2]
                        nop = mybir.InstEventSemaphore(
                            name=f"{inst.name}_ws{k}", ins=[], outs=[]
                        )
                        nop.engine = inst.engine
                        nop.sync_info = bass_rust.SyncInfo(on_wait=pair, on_update=[])
                        new_insts.append(nop)
                        n_split += 1
                    insts[i:i] = new_insts
                    i += len(new_insts)
                i += 1
    return n_split


def _build():
    if "nc" in _CACHE:
        return _CACHE["nc"]
    _ensure_paths()
    import bass_rust
    import concourse.bass as bass
    import concourse.mybir as mybir
    import concourse.tile as tile

    f32 = mybir.dt.float32
    f32r = mybir.dt.float32r
    bf16 = mybir.dt.bfloat16
    Exp = mybir.ActivationFunctionType.Exp

    nc = bass.Bass(trn_type="TRN2", target_bir_lowering=False, debug=False)

    xT_d = nc.dram_tensor("xT", [C, N], f32r, kind="ExternalInput").ap()
    wqk_d = nc.dram_tensor("w_qk", [C, 2 * DIM], f32r, kind="ExternalInput").ap()
    wv_d = nc.dram_tensor("w_v", [C, DIM], f32r, kind="ExternalInput").ap()
    wo_d = nc.dram_tensor("w_out", [DIM, DIM], f32r, kind="ExternalInput").ap()
    b_d = nc.dram_tensor("b_out", [DIM], f32r, kind="ExternalInput").ap()
    out_d = nc.dram_tensor("out", [N, DIM], f32, kind="ExternalOutput").ap()
    den_d = nc.dram_tensor("den_scratch", [8, N], f32).ap()
    rden_d = nc.dram_tensor("rden_scratch", [8, N], f32).ap()

    with tile.TileContext(nc) as tc:
        with (
            tc.tile_pool(name="wp", bufs=1) as wp,
            tc.tile_pool(name="xp", bufs=1) as xp,
            tc.tile_pool(name="qkp", bufs=4) as qkp,
            tc.tile_pool(name="vp", bufs=8) as vp,
            tc.tile_pool(name="ptp", bufs=28) as ptp,
            tc.tile_pool(name="otp", bufs=4) as otp,
            tc.tile_pool(name="dnp", bufs=4) as dnp,
            tc.tile_pool(name="bcp", bufs=4) as bcp,
            tc.tile_pool(name="avp", bufs=4) as avp,
            tc.tile_pool(name="yp", bufs=2) as yp,
            tc.tile_pool(name="psS", bufs=2, space="PSUM") as psS,
            tc.tile_pool(name="psAV", bufs=3, space="PSUM") as psAV,
            tc.tile_pool(name="psP", bufs=1, space="PSUM") as psP,
        ):
            # ---- input loads (sync queue) ----
            # first wave: ch0 halves of xT + the qk0/qk1 weight columns, so
            # the lead-in projection matmuls start as early as possible; the
            # remainder arrives while the lead-in runs.
            xT, wqk = [], []
            for ct in range(4):
                t = xp.tile([128, N], f32r, tag=f"xT{ct}", name=f"xT{ct}")
                nc.sync.dma_start(
                    out=t[:, 0:512], in_=xT_d[ct * 128 : (ct + 1) * 128, 0:512]
                )
                xT.append(t)
                t = wp.tile([128, 2 * DIM], f32r, tag=f"wqk{ct}", name=f"wqk{ct}")
                nc.sync.dma_start(
                    out=t[:, 0:256], in_=wqk_d[ct * 128 : (ct + 1) * 128, 0:256]
                )
                wqk.append(t)
            for ct in range(4):
                nc.sync.dma_start(
                    out=xT[ct][:, 512:1024],
                    in_=xT_d[ct * 128 : (ct + 1) * 128, 512:1024],
                )
                nc.sync.dma_start(
                    out=wqk[ct][:, 256 : 2 * DIM],
                    in_=wqk_d[ct * 128 : (ct + 1) * 128, 256 : 2 * DIM],
                )
            wv = []
            for ct in range(4):
                t = wp.tile([128, DIM], f32r, tag=f"wv{ct}", name=f"wv{ct}")
                nc.sync.dma_start(out=t[:], in_=wv_d[ct * 128 : (ct + 1) * 128, :])
                wv.append(t)
            wo = []
            for p4 in range(4):
                t = wp.tile([128, DIM], f32r, tag=f"wo{p4}", name=f"wo{p4}")
                nc.sync.dma_start(out=t[:], in_=wo_d[p4 * 128 : (p4 + 1) * 128, :])
                wo.append(t)
            # bias as a [1, 512] row: added to the final projection via an
            # extra K=1 matmul against an all-ones row
            b_sb = wp.tile([1, DIM], f32r, tag="bb", name="b_sb")
            b_src = bass.AP(tensor=b_d.tensor, offset=b_d.offset, ap=[[0, 1]] + list(b_d.ap))
            nc.sync.dma_start(out=b_sb[:], in_=b_src)
            ones_sb = wp.tile([1, 128], f32r, tag="ones", name="ones_sb")
            nc.vector.memset(ones_sb[:].bitcast(mybir.dt.uint32), 1065353216)

            qk = {}  # dt -> SBUF tile [128, N] f32r
            v_sb = {}  # jt -> SBUF tile [128, 6, DH+1] bf16 (pairs 0-2)
            v_sb3 = {}  # jt -> SBUF tile [128, 2, 2*DH] bf16 (pair 3, widened)
            pt = {}  # (p, jt, w) -> SBUF tile [128, N] bf16
            av = {}  # (p, ic, w) -> PSUM tile [65, 512]
            ot = {}  # p -> SBUF tile [128, N] f32r

            def qk_leadin():
                """qk tiles 0 and 1, interleaved per-ch accumulation groups
                with evacuations chasing each group, so the first score
                matmuls can start as early as possible. Uses the score PSUM
                pool (idle during lead-in)."""
                ps = {}
                for dt in range(2):
                    ps[dt] = psS.tile([128, N], f32, tag="s", name=f"qkps{dt}")
                    qk[dt] = qkp.tile([128, N], f32r, tag="qk", name=f"qk{dt}")
                for ch in range(2):
                    for dt in range(2):
                        for ct in range(4):
                            nc.tensor.matmul(
                                ps[dt][:, ch * 512 : (ch + 1) * 512],
                                wqk[ct][:, dt * 128 : (dt + 1) * 128],
                                xT[ct][:, ch * 512 : (ch + 1) * 512],
                                start=(ct == 0),
                                stop=(ct == 3),
                            )
                        nc.vector.tensor_copy(
                            qk[dt][:, ch * 512 : (ch + 1) * 512],
                            ps[dt][:, ch * 512 : (ch + 1) * 512],
                        )

            def mk_qkt_chunk(dt, part):
                """one ch-half of one d-tile of the qk projection (4 mms +
                evacuation of that half; each half gets its own 1-bank PSUM)"""

                def f():
                    if part == 0:
                        qk[dt] = qkp.tile([128, N], f32r, tag="qk", name=f"qk{dt}")
                    ch = part
                    ps = psP.tile([128, 512], f32, tag="p", name=f"qkps{dt}{ch}")
                    for ct in range(4):
                        nc.tensor.matmul(
                            ps[:],
                            wqk[ct][:, dt * 128 : (dt + 1) * 128],
                            xT[ct][:, ch * 512 : (ch + 1) * 512],
                            start=(ct == 0),
                            stop=(ct == 3),
                        )
                    nc.vector.tensor_copy(
                        qk[dt][:, ch * 512 : (ch + 1) * 512], ps[:]
                    )

                return f

            def mk_vpass_chunk(jt):
                def f():
                    # pairs 0-2: per-head stationary [128 j, v_h | ones-col];
                    # the ones column emits the softmax denominator as PSUM
                    # row 64 for free.
                    vt = vp.tile([128, 6, DH + 1], bf16, tag="v", name=f"v{jt}")
                    nc.vector.memset(
                        vt[:, :, DH : DH + 1].bitcast(mybir.dt.uint16), 16256
                    )
                    # pair 3 (heads 6,7): widened stationary [128 j, v_h (64)
                    # | ones (64)] so the AV matmul replicates the denominator
                    # across output partitions 64-127 (output partitions are
                    # free) -> the tail reciprocal runs straight from PSUM
                    # with no DMA bounce on the critical tail.
                    vt3 = vp.tile([128, 2, 2 * DH], bf16, tag="v3", name=f"v3_{jt}")
                    nc.vector.memset(
                        vt3[:, :, DH : 2 * DH].bitcast(mybir.dt.uint16), 16256
                    )
                    ps = psAV.tile([128, DIM], f32, tag="av", name=f"vps{jt}")
                    for ct in range(4):
                        nc.tensor.matmul(
                            ps[:],
                            xT[ct][:, jt * 128 : (jt + 1) * 128],
                            wv[ct][:],
                            start=(ct == 0),
                            stop=(ct == 3),
                        )
                    nc.vector.tensor_copy(
                        vt[:, :, 0:DH],
                        ps[:, 0 : 6 * DH].rearrange("p (h e) -> p h e", h=6),
                    )
                    nc.vector.tensor_copy(
                        vt3[:, :, 0:DH],
                        ps[:, 6 * DH :].rearrange("p (h e) -> p h e", h=2),
                    )
                    v_sb[jt] = vt
                    v_sb3[jt] = vt3

                return f

            def mk_av_chunk(p, ic, w, jp):
                """two AV matmuls (jt = 2*jp, 2*jp+1) accumulating into
                av[(p, ic, w)] [65, 512]"""

                def f():
                    if jp == 0:
                        # pair 3's ic1 accumulators borrow the score pool
                        # (free after the last exp) so the tail AV runs with
                        # no norm-chain wait
                        pool, tg = (psS, "s") if (p == 3 and ic == 1) else (psAV, "av")
                        rows = 128 if p == 3 else DH + 1
                        av[(p, ic, w)] = pool.tile(
                            [rows, 512], f32, tag=tg, name=f"av{p}_{ic}_{w}"
                        )
                    t = av[(p, ic, w)]
                    for jt in (2 * jp, 2 * jp + 1):
                        stat = (
                            v_sb3[jt][:, w, :] if p == 3 else v_sb[jt][:, 2 * p + w, :]
                        )
                        nc.tensor.matmul(
                            t[:],
                            stat,
                            pt[(p, jt, w)][:, ic * 512 : (ic + 1) * 512],
                            start=(jt == 0),
                            stop=(jt == 7),
                        )

                return f

            def mk_norm_chunk(p, ic):
                """denominator reciprocal (DVE) + partition broadcast (PE
                ones-matmul into PSUM) + normalize into ot[p] (DVE)"""

                def f():
                    if p == 3:
                        # tail path, fully on-chip: den is replicated across
                        # av PSUM rows 64-127 (widened-v stationary), so the
                        # reciprocal runs 64-lane wide straight from PSUM
                        # (4us on DVE, but zero DMA latency on the tail)
                        rc = [
                            bcp.tile([64, 512], f32, tag=f"bc{w}", name=f"rc3_{ic}_{w}")
                            for w in range(2)
                        ]
                        for w in range(2):
                            nc.vector.reciprocal(
                                rc[w][:], av[(p, ic, w)][DH : 2 * DH, :]
                            )
                        for w in range(2):
                            nc.vector.tensor_mul(
                                ot[p][w * 64 : (w + 1) * 64, ic * 512 : (ic + 1) * 512],
                                av[(p, ic, w)][0:DH, :],
                                rc[w][:],
                            )
                        return
                    # pairs 0-2 (front half): evacuate av to SBUF immediately
                    # (frees the PSUM accumulator ~1.4us after the last AV
                    # matmul), then run the den transpose bounce through DRAM
                    # entirely off the critical path: store row -> load
                    # [128, 8] transposed -> reciprocal (8 free elems) ->
                    # store -> stride-0 broadcast load. The normalize muls are
                    # deferred a full window (mk_norm_muls) so the long DMA
                    # chain never head-blocks the DVE queue.
                    row = 2 * p + ic
                    avs = [
                        avp.tile([DH + 1, 512], f32, tag=f"avs{w}", name=f"avs{p}{ic}{w}")
                        for w in range(2)
                    ]
                    for w in range(2):
                        nc.vector.tensor_copy(avs[w][:], av[(p, ic, w)][:])
                    for w in range(2):
                        nc.sync.dma_start(
                            out=den_d[row : row + 1, w * 512 : (w + 1) * 512],
                            in_=avs[w][DH : DH + 1, :],
                        )
                    dT = dnp.tile([128, 8], f32, tag="dT", name=f"dT{p}_{ic}")
                    dT_src = bass.AP(
                        tensor=den_d.tensor, offset=row * N, ap=[[1, 128], [128, 8]]
                    )
                    nc.sync.dma_start(out=dT[:], in_=dT_src)
                    rT = dnp.tile([128, 8], f32, tag="rT", name=f"rT{p}_{ic}")
                    nc.vector.reciprocal(rT[:], dT[:])
                    rT_dst = bass.AP(
                        tensor=rden_d.tensor, offset=row * N, ap=[[1, 128], [128, 8]]
                    )
                    nc.sync.dma_start(out=rT_dst, in_=rT[:])
                    bcs = [
                        bcp.tile([64, 512], f32, tag=f"bc{w}", name=f"bc{p}_{ic}_{w}")
                        for w in range(2)
                    ]
                    for w in range(2):
                        bc_src = bass.AP(
                            tensor=rden_d.tensor,
                            offset=row * N + w * 512,
                            ap=[[0, 64], [1, 512]],
                        )
                        nc.sync.dma_start(out=bcs[w][:], in_=bc_src)
                    _norm_state[(p, ic)] = (avs, bcs)

                return f

            _norm_state = {}

            def mk_norm_muls(p, ic):
                """deferred normalize multiplies for pairs 0-2 (emitted one
                window after the norm front, when the DMA chain is done)"""

                def f():
                    avs, bcs = _norm_state[(p, ic)]
                    for w in range(2):
                        nc.vector.tensor_mul(
                            ot[p][w * 64 : (w + 1) * 64, ic * 512 : (ic + 1) * 512],
                            avs[w][0:DH, :],
                            bcs[w][:],
                        )

                return f

            def window(p, bg):
                """phase A of pair p: 16 half-slots of (2 QK mms + exp), with
                background chunks distributed between them"""
                ot[p] = otp.tile([128, N], f32r, tag="ot", name=f"ot{p}")
                QT, KT = qk[2 * p], qk[2 * p + 1]
                n = len(bg)
                k = 0
                for hs in range(16):
                    jt, w = hs // 2, hs % 2
                    s = psS.tile([128, N], f32, tag="s", name=f"s{p}_{jt}_{w}")
                    lo, hi = w * 64, (w + 1) * 64
                    for ch in range(2):
                        nc.tensor.matmul(
                            s[:, ch * 512 : (ch + 1) * 512],
                            KT[lo:hi, jt * 128 : (jt + 1) * 128],
                            QT[lo:hi, ch * 512 : (ch + 1) * 512],
                            start=True,
                            stop=True,
                        )
                    t = ptp.tile([128, N], bf16, tag="pt", name=f"pt{p}_{jt}_{w}")
                    nc.scalar.activation(t[:], s[:], Exp, scale=SCALE)
                    pt[(p, jt, w)] = t
                    tgt = ((hs + 1) * n) // 16
                    while k < tgt:
                        bg[k]()
                        k += 1
                while k < n:
                    bg[k]()
                    k += 1

            def b_phase_chunks(p):
                chunks = []
                for ic in range(2):
                    for w in range(2):
                        for jp in range(4):
                            chunks.append(mk_av_chunk(p, ic, w, jp))
                    chunks.append(mk_norm_chunk(p, ic))
                return chunks

            def b_muls_chunks(p):
                return [mk_norm_muls(p, ic) for ic in range(2)]

            # ---- lead-in: first pair's projection tiles ----
            qk_leadin()

            # ---- pair 0: v-pass + next pair's projections underneath ----
            bg0 = [mk_vpass_chunk(jt) for jt in range(8)]
            bg0 += [mk_qkt_chunk(2, 0), mk_qkt_chunk(2, 1)]
            bg0 += [mk_qkt_chunk(3, 0), mk_qkt_chunk(3, 1)]
            window(0, bg0)

            # ---- pairs 1..3: previous pair's AV/norm-fronts + projections;
            # normalize muls deferred one further window ----
            for p in range(1, 4):
                bg = []
                if p >= 2:
                    bg += b_muls_chunks(p - 2)
                if p < 3:
                    bg += [mk_qkt_chunk(2 * p + 2, 0), mk_qkt_chunk(2 * p + 2, 1)]
                    bg += [mk_qkt_chunk(2 * p + 3, 0), mk_qkt_chunk(2 * p + 3, 1)]
                bg += b_phase_chunks(p - 1)
                window(p, bg)

            # ---- tail: pair 2 muls, B phase of pair 3, final ----
            for ch_fn in b_muls_chunks(2) + b_phase_chunks(3):
                ch_fn()

            # ---- final projection (bias folded in as a K=1 matmul against
            # an all-ones row); evacuation on the now-idle scalar engine ----
            for it in range(8):
                fps = psAV.tile([128, DIM], f32, tag="av", name=f"fps{it}")
                for p4 in range(4):
                    nc.tensor.matmul(
                        fps[:],
                        ot[p4][:, it * 128 : (it + 1) * 128],
                        wo[p4][:],
                        start=(p4 == 0),
                        stop=False,
                    )
                nc.tensor.matmul(
                    fps[:], ones_sb[:], b_sb[:], start=False, stop=True
                )
                y = yp.tile([128, DIM], f32, tag="y", name=f"y{it}")
                nc.scalar.copy(y[:], fps[:])
                eng = nc.sync if it % 2 == 0 else nc.scalar
                eng.dma_start(out=out_d[it * 128 : (it + 1) * 128, :], in_=y[:])

    _split_excess_waits(nc, mybir, bass_rust)
    _CACHE["nc"] = nc
    return nc


def _prep_inputs(inputs):
    x = np.ascontiguousarray(inputs["x"], dtype=np.float32)
    w_qkv = np.ascontiguousarray(inputs["w_qkv"], dtype=np.float32)
    w_out = np.ascontiguousarray(inputs["w_out"], dtype=np.float32)
    b_out = np.ascontiguousarray(inputs["b_out"], dtype=np.float32)

    # per-head slices of the fused qkv weight
    wq = [w_qkv[:, h * 192 : h * 192 + 64] for h in range(HEAD)]
    wk = [w_qkv[:, h * 192 + 64 : h * 192 + 128] for h in range(HEAD)]
    wvl = [w_qkv[:, h * 192 + 128 : h * 192 + 192] for h in range(HEAD)]
    # pair-banded column order: [q0 q1 k0 k1 | q2 q3 k2 k3 | ...]
    blocks = []
    for p in range(4):
        blocks += [wq[2 * p], wq[2 * p + 1], wk[2 * p], wk[2 * p + 1]]
    w_qk = np.ascontiguousarray(np.concatenate(blocks, axis=1))
    w_v = np.ascontiguousarray(np.concatenate(wvl, axis=1))

    in_maps = []
    for i in range(N_CORES):
        xT = np.ascontiguousarray(x[i].reshape(N, C).T)
        in_maps.append(
            {"xT": xT, "w_qk": w_qk, "w_v": w_v, "w_out": w_out, "b_out": b_out}
        )
    return in_maps


def _run(inputs, trace=False):
    _ensure_paths()
    import os

    if trace:
        os.environ.pop("BASS_NEVER_TRACE", None)
    else:
        os.environ["BASS_NEVER_TRACE"] = "1"
    from concourse import bass_utils

    nc = _build()
    in_maps = _prep_inputs(inputs)
    res = bass_utils.run_bass_kernel_spmd(
        nc, in_maps, core_ids=list(range(N_CORES)), trace=trace
    )
    out = np.stack(
        [res.results[i]["out"].reshape(32, 32, DIM) for i in range(N_CORES)]
    ).astype(np.float32)
    return out, res


def kernel(**inputs):
    out, _ = _run(inputs, trace=False)
    return out


# revision 46
# speedup vs baseline: 1.1584x; 1.1584x over previous
"""Trainium2 Bass kernel for nn_Attention_85796266705382.

Reference computation (per batch element, b=8, HEAD=8, n=32*32=1024, c=dim=512):
    qkv = x @ w_qkv                      # (n, 1536), per-head interleaved [q|k|v] x 64
    q,k,v per head (n, 64)
    attn = softmax(q @ k.T * 8**-0.5)    # scale uses FULL batch size (reference quirk)
    out  = attn @ v                      # (n, 64) per head -> (n, 512)
    y    = out @ w_out + b_out           # (n, 512)

Sharding: pure data-parallel over batch - one batch element per NeuronCore (8 cores).

Per-core design (v2): the kernel is scheduled as a continuous software pipeline
around the two nearly co-critical engines: PE (~82us of matmul rows at 2.4GHz)
and ACT (64 exps of [128,1024] ~ 69us). The pipelined unit is one (j-tile, head)
half-slot: 2 QK matmuls -> exp -> pt tile. All other PE work (QKV projections,
v-pass, AV matmuls of the PREVIOUS pair, final projection chunks) is emitted in
small chunks between the QK matmuls so the exp stream never starves and PE never
idles waiting on exp.

Key layout/scheduling choices:
  * qkT [1024, n] = w_qk.T @ xT with host-permuted pair-banded columns
    [q_h0 q_h1 k_h0 k_h1 | ...] -> each 128-row tile is a head-PAIR band.
  * scores (transposed) sT_h [j, i] per (pair, jt, head): 2 matmuls (i-chunks of
    512), one [128,1024] exp on ACT (folds the 8**-0.5 scale), bf16 pt output.
  * v natural [n, 512] via f32r matmuls (no bf16 cast of xT needed); stored
    bf16 with 65-column per-head pitch, col 64 = ones -> AV matmul emits the
    softmax denominators for free in PSUM row 64.
  * AV per pair split into two i-chunk phases of [65,512] accumulators so only
    2 PSUM banks are held: scores 2x[128,1024] (4 banks) + AV 2x[65,512] (2) +
    projection transient [128,1024] (2) = 8 banks exactly.
  * normalization fully on-chip: DVE reciprocal of PSUM den row -> gpsimd
    partition_broadcast -> DVE multiply into ot[p] (no DRAM bounce).
  * final projection y = ot.T @ w_out accumulated over pairs in PSUM at the
    end; bias add on DVE; stores alternate sync/scalar DMA queues.
"""

import numpy as np


def _ensure_paths():
    import sys

    try:
        import concourse.bass  # noqa: F401

        return
    except ImportError:
        pass
    for p in ("/opt/trn_rl_repo", "/root/.axon_site/_ro/trn_rl_repo"):
        if p not in sys.path:
            sys.path.append(p)
    import concourse.bass  # noqa: F401


HEAD = 8
B = 8
N = 1024  # tokens per batch element (32*32)
C = 512  # channels
DIM = 512
DH = 64
SCALE = float(B) ** -0.5  # reference scales by batch size, reproduced faithfully
N_CORES = 8

_CACHE = {}


def _split_excess_waits(nc, mybir, bass_rust):
    """walrus in this container accepts 1 sync wait per instruction (2 for
    EventSemaphore); Tile sometimes attaches more. Hoist the excess onto fresh
    same-engine NoOps inserted just before the over-capacity instruction."""
    n_split = 0
    for fn in nc.m.functions:
        for bb in fn.blocks:
            insts = bb.instructions
            i = 0
            while i < len(insts):
                inst = insts[i]
                si = inst.sync_info
                cap = 2 if isinstance(inst, mybir.InstEventSemaphore) else 1
                if si is not None and len(si.on_wait) > cap:
                    extra = list(si.on_wait[cap:])
                    del si.on_wait[cap:]
                    new_insts = []
                    for k in range(0, len(extra), 2):
                        pair = extra[k : k + 2]
                        nop = mybir.InstEventSemaphore(
                            name=f"{inst.name}_ws{k}", ins=[], outs=[]
                        )
                        nop.engine = inst.engine
                        nop.sync_info = bass_rust.SyncInfo(on_wait=pair, on_update=[])
                        new_insts.append(nop)
                        n_split += 1
                    insts[i:i] = new_insts
                    i += len(new_insts)
                i += 1
    return n_split


def _build():
    if "nc" in _CACHE:
        return _CACHE["nc"]
    _ensure_paths()
    import bass_rust
    import concourse.bass as bass
    import concourse.mybir as mybir
    import concourse.tile as tile

    f32 = mybir.dt.float32
    f32r = mybir.dt.float32r
    bf16 = mybir.dt.bfloat16
    Exp = mybir.ActivationFunctionType.Exp

    nc = bass.Bass(trn_type="TRN2", target_bir_lowering=False, debug=False)

    xT_d = nc.dram_tensor("xT", [C, N], f32r, kind="ExternalInput").ap()
    wqk_d = nc.dram_tensor("w_qk", [C, 2 * DIM], f32r, kind="ExternalInput").ap()
    wv_d = nc.dram_tensor("w_v", [C, DIM], f32r, kind="ExternalInput").ap()
    wo_d = nc.dram_tensor("w_out", [DIM, DIM], f32r, kind="ExternalInput").ap()
    b_d = nc.dram_tensor("b_out", [DIM], f32r, kind="ExternalInput").ap()
    out_d = nc.dram_tensor("out", [N, DIM], f32, kind="ExternalOutput").ap()
    den_d = nc.dram_tensor("den_scratch", [8, N], f32).ap()
    rden_d = nc.dram_tensor("rden_scratch", [8, N], f32).ap()

    with tile.TileContext(nc) as tc:
        with (
            tc.tile_pool(name="wp", bufs=1) as wp,
            tc.tile_pool(name="xp", bufs=1) as xp,
            tc.tile_pool(name="qkp", bufs=4) as qkp,
            tc.tile_pool(name="vp", bufs=8) as vp,
            tc.tile_pool(name="ptp", bufs=28) as ptp,
            tc.tile_pool(name="otp", bufs=4) as otp,
            tc.tile_pool(name="dnp", bufs=4) as dnp,
            tc.tile_pool(name="bcp", bufs=4) as bcp,
            tc.tile_pool(name="avp", bufs=4) as avp,
            tc.tile_pool(name="yp", bufs=2) as yp,
            tc.tile_pool(name="psS", bufs=2, space="PSUM") as psS,
            tc.tile_pool(name="psAV", bufs=3, space="PSUM") as psAV,
            tc.tile_pool(name="psP", bufs=1, space="PSUM") as psP,
        ):
            # ---- input loads (sync queue) ----
            # first wave: ch0 halves of xT + the qk0/qk1 weight columns, so
            # the lead-in projection matmuls start as early as possible; the
            # remainder arrives while the lead-in runs.
            xT, wqk = [], []
            for ct in range(4):
                t = xp.tile([128, N], f32r, tag=f"xT{ct}", name=f"xT{ct}")
                nc.sync.dma_start(
                    out=t[:, 0:512], in_=xT_d[ct * 128 : (ct + 1) * 128, 0:512]
                )
                xT.append(t)
                t = wp.tile([128, 2 * DIM], f32r, tag=f"wqk{ct}", name=f"wqk{ct}")
                nc.sync.dma_start(
                    out=t[:, 0:256], in_=wqk_d[ct * 128 : (ct + 1) * 128, 0:256]
                )
                wqk.append(t)
            for ct in range(4):
                nc.sync.dma_start(
                    out=xT[ct][:, 512:1024],
                    in_=xT_d[ct * 128 : (ct + 1) * 128, 512:1024],
                )
                nc.sync.dma_start(
                    out=wqk[ct][:, 256 : 2 * DIM],
                    in_=wqk_d[ct * 128 : (ct + 1) * 128, 256 : 2 * DIM],
                )
            wv = []
            for ct in range(4):
                t = wp.tile([128, DIM], f32r, tag=f"wv{ct}", name=f"wv{ct}")
                nc.sync.dma_start(out=t[:], in_=wv_d[ct * 128 : (ct + 1) * 128, :])
                wv.append(t)
            wo = []
            for p4 in range(4):
                t = wp.tile([128, DIM], f32r, tag=f"wo{p4}", name=f"wo{p4}")
                nc.sync.dma_start(out=t[:], in_=wo_d[p4 * 128 : (p4 + 1) * 128, :])
                wo.append(t)
            # bias as a [1, 512] row: added to the final projection via an
            # extra K=1 matmul against an all-ones row
            b_sb = wp.tile([1, DIM], f32r, tag="bb", name="b_sb")
            b_src = bass.AP(tensor=b_d.tensor, offset=b_d.offset, ap=[[0, 1]] + list(b_d.ap))
            nc.sync.dma_start(out=b_sb[:], in_=b_src)
            ones_sb = wp.tile([1, 128], f32r, tag="ones", name="ones_sb")
            nc.vector.memset(ones_sb[:].bitcast(mybir.dt.uint32), 1065353216)

            qk = {}  # dt -> SBUF tile [128, N] f32r
            v_sb = {}  # jt -> SBUF tile [128, 6, DH+1] bf16 (pairs 0-2)
            v_sb3 = {}  # jt -> SBUF tile [128, 2, 2*DH] bf16 (pair 3, widened)
            pt = {}  # (p, jt, w) -> SBUF tile [128, N] bf16
            av = {}  # (p, ic, w) -> PSUM tile [65, 512]
            ot = {}  # p -> SBUF tile [128, N] f32r

            def qk_leadin():
                """qk tiles 0 and 1, interleaved per-ch accumulation groups
                with evacuations chasing each group, so the first score
                matmuls can start as early as possible. Uses the score PSUM
                pool (idle during lead-in)."""
                ps = {}
                for dt in range(2):
                    ps[dt] = psS.tile([128, N], f32, tag="s", name=f"qkps{dt}")
                    qk[dt] = qkp.tile([128, N], f32r, tag="qk", name=f"qk{dt}")
                for ch in range(2):
                    for dt in range(2):
                        for ct in range(4):
                            nc.tensor.matmul(
                                ps[dt][:, ch * 512 : (ch + 1) * 512],
                                wqk[ct][:, dt * 128 : (dt + 1) * 128],
                                xT[ct][:, ch * 512 : (ch + 1) * 512],
                                start=(ct == 0),
                                stop=(ct == 3),
                            )
                        nc.vector.tensor_copy(
                            qk[dt][:, ch * 512 : (ch + 1) * 512],
                            ps[dt][:, ch * 512 : (ch + 1) * 512],
                        )

            def mk_qkt_chunk(dt, part):
                """one ch-half of one d-tile of the qk projection (4 mms +
                evacuation of that half; each half gets its own 1-bank PSUM)"""

                def f():
                    if part == 0:
                        qk[dt] = qkp.tile([128, N], f32r, tag="qk", name=f"qk{dt}")
                    ch = part
                    ps = psP.tile([128, 512], f32, tag="p", name=f"qkps{dt}{ch}")
                    for ct in range(4):
                        nc.tensor.matmul(
                            ps[:],
                            wqk[ct][:, dt * 128 : (dt + 1) * 128],
                            xT[ct][:, ch * 512 : (ch + 1) * 512],
                            start=(ct == 0),
                            stop=(ct == 3),
                        )
                    nc.vector.tensor_copy(
                        qk[dt][:, ch * 512 : (ch + 1) * 512], ps[:]
                    )

                return f

            def mk_vpass_chunk(jt):
                def f():
                    # pairs 0-2: per-head stationary [128 j, v_h | ones-col];
                    # the ones column emits the softmax denominator as PSUM
                    # row 64 for free.
                    vt = vp.tile([128, 6, DH + 1], bf16, tag="v", name=f"v{jt}")
                    nc.vector.memset(
                        vt[:, :, DH : DH + 1].bitcast(mybir.dt.uint16), 16256
                    )
                    # pair 3 (heads 6,7): widened stationary [128 j, v_h (64)
                    # | ones (64)] so the AV matmul replicates the denominator
                    # across output partitions 64-127 (output partitions are
                    # free) -> the tail reciprocal runs straight from PSUM
                    # with no DMA bounce on the critical tail.
                    vt3 = vp.tile([128, 2, 2 * DH], bf16, tag="v3", name=f"v3_{jt}")
                    nc.vector.memset(
                        vt3[:, :, DH : 2 * DH].bitcast(mybir.dt.uint16), 16256
                    )
                    ps = psAV.tile([128, DIM], f32, tag="av", name=f"vps{jt}")
                    for ct in range(4):
                        nc.tensor.matmul(
                            ps[:],
                            xT[ct][:, jt * 128 : (jt + 1) * 128],
                            wv[ct][:],
                            start=(ct == 0),
                            stop=(ct == 3),
                        )
                    nc.vector.tensor_copy(
                        vt[:, :, 0:DH],
                        ps[:, 0 : 6 * DH].rearrange("p (h e) -> p h e", h=6),
                    )
                    nc.vector.tensor_copy(
                        vt3[:, :, 0:DH],
                        ps[:, 6 * DH :].rearrange("p (h e) -> p h e", h=2),
                    )
                    v_sb[jt] = vt
                    v_sb3[jt] = vt3

                return f

            def mk_av_chunk(p, ic, w, jp):
                """two AV matmuls (jt = 2*jp, 2*jp+1) accumulating into
                av[(p, ic, w)] [65, 512]"""

                def f():
                    if jp == 0:
                        # pair 3's ic1 accumulators borrow the score pool
                        # (free after the last exp) so the tail AV runs with
                        # no norm-chain wait
                        pool, tg = (psS, "s") if (p == 3 and ic == 1) else (psAV, "av")
                        rows = 128 if p == 3 else DH + 1
                        av[(p, ic, w)] = pool.tile(
                            [rows, 512], f32, tag=tg, name=f"av{p}_{ic}_{w}"
                        )
                    t = av[(p, ic, w)]
                    for jt in (2 * jp, 2 * jp + 1):
                        stat = (
                            v_sb3[jt][:, w, :] if p == 3 else v_sb[jt][:, 2 * p + w, :]
                        )
                        nc.tensor.matmul(
                            t[:],
                            stat,
                            pt[(p, jt, w)][:, ic * 512 : (ic + 1) * 512],
                            start=(jt == 0),
                            stop=(jt == 7),
                        )

                return f

            def mk_norm_chunk(p, ic):
                """denominator reciprocal (DVE) + partition broadcast (PE
                ones-matmul into PSUM) + normalize into ot[p] (DVE)"""

                def f():
                    if p == 3:
                        # tail path, fully on-chip: den is replicated across
                        # av PSUM rows 64-127 (widened-v stationary), so the
                        # reciprocal runs 64-lane wide straight from PSUM
                        # (4us on DVE, but zero DMA latency on the tail)
                        rc = [
                            bcp.tile([64, 512], f32, tag=f"bc{w}", name=f"rc3_{ic}_{w}")
                            for w in range(2)
                        ]
                        for w in range(2):
                            nc.vector.reciprocal(
                                rc[w][:], av[(p, ic, w)][DH : 2 * DH, :]
                            )
                        for w in range(2):
                            nc.vector.tensor_mul(
                                ot[p][w * 64 : (w + 1) * 64, ic * 512 : (ic + 1) * 512],
                                av[(p, ic, w)][0:DH, :],
                                rc[w][:],
                            )
                        return
                    # pairs 0-2 (front half): evacuate av to SBUF immediately
                    # (frees the PSUM accumulator ~1.4us after the last AV
                    # matmul), then run the den transpose bounce through DRAM
                    # entirely off the critical path: store row -> load
                    # [128, 8] transposed -> reciprocal (8 free elems) ->
                    # store -> stride-0 broadcast load. The normalize muls are
                    # deferred a full window (mk_norm_muls) so the long DMA
                    # chain never head-blocks the DVE queue.
                    row = 2 * p + ic
                    avs = [
                        avp.tile([DH + 1, 512], f32, tag=f"avs{w}", name=f"avs{p}{ic}{w}")
                        for w in range(2)
                    ]
                    for w in range(2):
                        nc.vector.tensor_copy(avs[w][:], av[(p, ic, w)][:])
                    for w in range(2):
                        nc.sync.dma_start(
                            out=den_d[row : row + 1, w * 512 : (w + 1) * 512],
                            in_=avs[w][DH : DH + 1, :],
                        )
                    dT = dnp.tile([128, 8], f32, tag="dT", name=f"dT{p}_{ic}")
                    dT_src = bass.AP(
                        tensor=den_d.tensor, offset=row * N, ap=[[1, 128], [128, 8]]
                    )
                    nc.sync.dma_start(out=dT[:], in_=dT_src)
                    rT = dnp.tile([128, 8], f32, tag="rT", name=f"rT{p}_{ic}")
                    nc.vector.reciprocal(rT[:], dT[:])
                    rT_dst = bass.AP(
                        tensor=rden_d.tensor, offset=row * N, ap=[[1, 128], [128, 8]]
                    )
                    nc.sync.dma_start(out=rT_dst, in_=rT[:])
                    bcs = [
                        bcp.tile([64, 512], f32, tag=f"bc{w}", name=f"bc{p}_{ic}_{w}")
                        for w in range(2)
                    ]
                    for w in range(2):
                        bc_src = bass.AP(
                            tensor=rden_d.tensor,
                            offset=row * N + w * 512,
                            ap=[[0, 64], [1, 512]],
                        )
                        nc.sync.dma_start(out=bcs[w][:], in_=bc_src)
                    _norm_state[(p, ic)] = (avs, bcs)

                return f

            _norm_state = {}

            def mk_norm_muls(p, ic):
                """deferred normalize multiplies for pairs 0-2 (emitted one
                window after the norm front, when the DMA chain is done)"""

                def f():
                    avs, bcs = _norm_state[(p, ic)]
                    for w in range(2):
                        nc.vector.tensor_mul(
                            ot[p][w * 64 : (w + 1) * 64, ic * 512 : (ic + 1) * 512],
                            avs[w][0:DH, :],
                            bcs[w][:],
                        )

                return f

            def window(p, bg):
                """phase A of pair p: 16 half-slots of (2 QK mms + exp), with
                background chunks distributed between them"""
                ot[p] = otp.tile([128, N], f32r, tag="ot", name=f"ot{p}")
                QT, KT = qk[2 * p], qk[2 * p + 1]
                n = len(bg)
                k = 0
                for hs in range(16):
                    jt, w = hs // 2, hs % 2
                    s = psS.tile([128, N], f32, tag="s", name=f"s{p}_{jt}_{w}")
                    lo, hi = w * 64, (w + 1) * 64
                    for ch in range(2):
                        nc.tensor.matmul(
                            s[:, ch * 512 : (ch + 1) * 512],
                            KT[lo:hi, jt * 128 : (jt + 1) * 128],
                            QT[lo:hi, ch * 512 : (ch + 1) * 512],
                            start=True,
                            stop=True,
                        )
                    t = ptp.tile([128, N], bf16, tag="pt", name=f"pt{p}_{jt}_{w}")
                    nc.scalar.activation(t[:], s[:], Exp, scale=SCALE)
                    pt[(p, jt, w)] = t
                    tgt = ((hs + 1) * n) // 16
                    while k < tgt:
                        bg[k]()
                        k += 1
                while k < n:
                    bg[k]()
                    k += 1

            def b_phase_chunks(p):
                chunks = []
                for ic in range(2):
                    for w in range(2):
                        for jp in range(4):
                            chunks.append(mk_av_chunk(p, ic, w, jp))
                    chunks.append(mk_norm_chunk(p, ic))
                return chunks

            def b_muls_chunks(p):
                return [mk_norm_muls(p, ic) for ic in range(2)]

            # ---- lead-in: first pair's projection tiles ----
            qk_leadin()

            # ---- pair 0: v-pass + next pair's projections underneath ----
            bg0 = [mk_vpass_chunk(jt) for jt in range(8)]
            bg0 += [mk_qkt_chunk(2, 0), mk_qkt_chunk(2, 1)]
            bg0 += [mk_qkt_chunk(3, 0), mk_qkt_chunk(3, 1)]
            window(0, bg0)

            # ---- pairs 1..3: previous pair's AV/norm-fronts + projections;
            # normalize muls deferred one further window (their DMA chains
            # get a full window, so they never head-block the DVE queue) ----
            for p in range(1, 4):
                bg = []
                if p < 3:
                    bg += [mk_qkt_chunk(2 * p + 2, 0), mk_qkt_chunk(2 * p + 2, 1)]
                    bg += [mk_qkt_chunk(2 * p + 3, 0), mk_qkt_chunk(2 * p + 3, 1)]
                bg += b_phase_chunks(p - 1)
                if p >= 2:
                    bg += b_muls_chunks(p - 2)
                window(p, bg)

            # ---- tail: B phase of pair 3 interleaved with pair-2 muls ----
            ch3 = b_phase_chunks(3)
            for ch_fn in ch3[0:9]:  # av ic0 + norm(3, ic0)
                ch_fn()
            mk_norm_muls(2, 0)()
            for ch_fn in ch3[9:]:  # av ic1 + norm(3, ic1)
                ch_fn()
            mk_norm_muls(2, 1)()

            # ---- final projection (bias folded in as a K=1 matmul against
            # an all-ones row); evacuation on the now-idle scalar engine ----
            for it in range(8):
                fps = psAV.tile([128, DIM], f32, tag="av", name=f"fps{it}")
                for p4 in range(4):
                    nc.tensor.matmul(
                        fps[:],
                        ot[p4][:, it * 128 : (it + 1) * 128],
                        wo[p4][:],
                        start=(p4 == 0),
                        stop=False,
                    )
                nc.tensor.matmul(
                    fps[:], ones_sb[:], b_sb[:], start=False, stop=True
                )
                y = yp.tile([128, DIM], f32, tag="y", name=f"y{it}")
                nc.scalar.copy(y[:], fps[:])
                eng = nc.sync if it % 2 == 0 else nc.scalar
                eng.dma_start(out=out_d[it * 128 : (it + 1) * 128, :], in_=y[:])

    _split_excess_waits(nc, mybir, bass_rust)
    _CACHE["nc"] = nc
    return nc


def _prep_inputs(inputs):
    x = np.ascontiguousarray(inputs["x"], dtype=np.float32)
    w_qkv = np.ascontiguousarray(inputs["w_qkv"], dtype=np.float32)
    w_out = np.ascontiguousarray(inputs["w_out"], dtype=np.float32)
    b_out = np.ascontiguousarray(inputs["b_out"], dtype=np.float32)

    # per-head slices of the fused qkv weight
    wq = [w_qkv[:, h * 192 : h * 192 + 64] for h in range(HEAD)]
    wk = [w_qkv[:, h * 192 + 64 : h * 192 + 128] for h in range(HEAD)]
    wvl = [w_qkv[:, h * 192 + 128 : h * 192 + 192] for h in range(HEAD)]
    # pair-banded column order: [q0 q1 k0 k1 | q2 q3 k2 k3 | ...]
    blocks = []
    for p in range(4):
        blocks += [wq[2 * p], wq[2 * p + 1], wk[2 * p], wk[2 * p + 1]]
    w_qk = np.ascontiguousarray(np.concatenate(blocks, axis=1))
    w_v = np.ascontiguousarray(np.concatenate(wvl, axis=1))

    in_maps = []
    for i in range(N_CORES):
        xT = np.ascontiguousarray(x[i].reshape(N, C).T)
        in_maps.append(
            {"xT": xT, "w_qk": w_qk, "w_v": w_v, "w_out": w_out, "b_out": b_out}
        )
    return in_maps


def _run(inputs, trace=False):
    _ensure_paths()
    import os

    if trace:
        os.environ.pop("BASS_NEVER_TRACE", None)
    else:
        os.environ["BASS_NEVER_TRACE"] = "1"
    from concourse import bass_utils

    nc = _build()
    in_maps = _prep_inputs(inputs)
    res = bass_utils.run_bass_kernel_spmd(
        nc, in_maps, core_ids=list(range(N_CORES)), trace=trace
    )
    out = np.stack(
        [res.results[i]["out"].reshape(32, 32, DIM) for i in range(N_CORES)]
    ).astype(np.float32)
    return out, res


def kernel(**inputs):
    out, _ = _run(inputs, trace=False)
    return out
